# revision 45
# baseline (speedup 1.0000x reference)
"""Trainium2 Bass kernel for nn_DenseGNOBlock (B=4, N=8192, C=64).

Reference computes, per batch b:
    q = x Wq^T + bq ; k = x Wk^T + bk ; v = x Wv^T + bv
    kernel = q k^T / sqrt(C) ; integral = kernel v / N
    out = gelu(x Ww^T + bw + integral)

No softmax, so the N x N kernel reassociates away completely. With
Gt = [x|1]^T [x|1] (65 x 65, per batch) and Wt* = [W* | b*]:
    S = k^T v = Wtk Gt Wtv^T
    out = gelu(x @ Mmat + c^T)        broadcast row bias c
    Mmat = Ww^T + a Wq^T S ;  c = bw + a S^T bq ;  a = 1/(sqrt(C) N)

Per core: one pass of x through the PE for Gt, a tiny matrix chain,
one pass for the output. Everything is exact fp32. The rest is layout
engineering:

- All weights/biases/identities/selectors ship as ONE packed
  [128, 386] input (host-prepared, transposed and ALPHA-folded where
  needed) -> a single DMA instead of a dozen small serialized ones.
  I128 doubles as e_top=[I;0], e_bot=[0;I], I64, and the row
  shift/selector matrices used to assemble the Mt variants.
- x rows are packed in "pair blocks" [1 | x_even | x_odd | 1] (130
  cols) so the x DMA lands 512B-contiguous per partition AND both
  matmuls of a column-packed G-pair get an augmented rhs for free
  (even: rhs=[1|x_e] -> [m_e|G_e], odd: rhs=[x_o|1] -> [G_o|m_o]),
  AND the forward transposes get augmented inputs for free
  ([1|x_e]^T has the ones row at row 0, [x_o|1]^T at row 64).
- Input DMAs alternate between the two HWDGE rings (SP via nc.sync,
  ACT via nc.scalar) so issue/transfer overhead pipelines.
- G/m accumulate column-packed (tile_position (0,0)/(0,64)) into
  separate PSUM banks (start=True clears has_written bank-wide), then
  fold top+bottom with matmuls against I128's column halves.
- The chain is host-shortened: T1 = Gt Wtv^T, then one matmul against
  the host-folded utq = [(a Wq^T Wtk)^T | a Wtk^T bq] plus one add of
  [Ww^T; bw] yields Mt_odd = [Mmat; c_row] directly; Mt_even =
  [c_row; Mmat] is one cyclic-row-shift matmul away.
- Forward transposes interleave with the G phase (they do not depend
  on the chain); final matmuls are K=65 augmented (bias included), so
  gelu reads PSUM and writes the output buffer directly -- no bias
  add, no transpose-back, no extra copies.

Sharding: 8 cores, core c -> batch b = c//2, half h = c%2. Each core
receives the full x_b (rotated so its own 4096 rows come first),
computes Gt over all of x_b (order-invariant), and writes its own half.
"""

import sys

for _p in ("/opt/trn_rl_repo", "/root/.axon_site/_ro/trn_rl_repo"):
    if _p not in sys.path:
        sys.path.append(_p)

import numpy as np
from contextlib import ExitStack

import concourse.bass as bass
import concourse.bacc as bacc
import concourse.mybir as mybir
import concourse.tile as tile
from concourse.bass_utils import run_bass_kernel_spmd

FP = mybir.dt.float32
FPR = mybir.dt.float32r
AF = mybir.ActivationFunctionType
MUL = mybir.AluOpType.mult
ADD = mybir.AluOpType.add

B, N, C = 4, 8192, 64
P = 128              # partitions
W = C + 1            # augmented width
NPAIR = N // (2 * P)  # 32 pair blocks per batch
BLK = 2 * C + 2      # 130 cols: [1 | x_even | x_odd | 1]
HPAIR = NPAIR // 2   # 16 own pair blocks
NCORES = 8
ALPHA = 1.0 / (np.sqrt(np.float32(C)) * np.float32(N))
DMA_GP = 2           # pair blocks per x dma_start (16 groups)
# packed weight layout (free offsets)
WPK_VT = 0           # [0:65, 0:64]    [Wv^T ; bv^T]
WPK_UTQ = 64         # [0:65, 64:129]  [(a Wq^T Wtk)^T | a Wtk^T bq]
WPK_WB = 129         # [0:65, 129:193] [Ww^T ; bw-row]
WPK_CYC = 193        # [0:65, 193:258] cyc[k,i] = (i == (k+1) mod 65)
WPK_ID = 258         # [:, 258:386]    I128
WPK_F = WPK_ID + P   # 386 total


def build_nc(act: str = "gelu") -> bass.Bass:
    act_fn = {"gelu": AF.Gelu, "identity": AF.Identity}[act]
    nc = bacc.Bacc("TRN2", target_bir_lowering=False, debug=False)

    x_d = nc.declare_dram_parameter("xb", [P, NPAIR * BLK], FP, isOutput=False)
    wpk_d = nc.declare_dram_parameter("wpk", [P, WPK_F], FP, isOutput=False)
    out_d = nc.declare_dram_parameter("out", [N // 2, C], FP, isOutput=True)

    with ExitStack() as ctx:
        tc = ctx.enter_context(tile.TileContext(nc))
        const = ctx.enter_context(tc.tile_pool(name="const", bufs=1))
        ps_g = ctx.enter_context(tc.tile_pool(name="ps_g", bufs=2, space="PSUM"))
        ps_big = ctx.enter_context(tc.tile_pool(name="ps_big", bufs=2, space="PSUM"))
        sb_t = ctx.enter_context(tc.tile_pool(name="sb_t", bufs=3))

        wpk = const.tile([P, WPK_F], FP)
        wvta = wpk[0:W, WPK_VT : WPK_VT + C]
        utq = wpk[0:W, WPK_UTQ : WPK_UTQ + W]
        wwbw = wpk[0:W, WPK_WB : WPK_WB + C]
        cyc = wpk[0:W, WPK_CYC : WPK_CYC + W]
        ident = wpk[:, WPK_ID : WPK_ID + P]
        e_top = ident[:, 0:C]            # [I64; 0]
        e_bot = ident[:, C:P]            # [0; I64]
        id64 = ident[0:C, 0:C]

        # --- stream x in: host-prebuilt pair blocks [1 | x_e | x_o | 1] ---
        # row(p, pair, j) = pair*256 + 2p + j; ones columns included by the
        # host so the transfers are fully contiguous per partition and the
        # early G matmuls wait on exactly one DMA lane; dual HWDGE rings
        xsb = const.tile([P, NPAIR, BLK], FP)
        xr = x_d[:].rearrange("p (b k) -> p b k", k=BLK)
        # a tiny pair-0 DMA goes absolutely first: every DMA pays a fixed
        # completion-receipt latency before its semaphore fires, so the
        # first PE matmul is gated by (first dma end + receipt) -- keep
        # that transfer as small as possible. Early groups ride the SP
        # ring (the ACT ring opens with a ~1.3us activation-table load).
        nc.sync.dma_start(out=xsb[:, 0:1, :], in_=xr[:, 0:1, :])
        nc.sync.dma_start(out=xsb[:, 1:2, :], in_=xr[:, 1:2, :])
        nc.sync.dma_start(out=wpk[:], in_=wpk_d[:])
        for g in range(1, NPAIR // DMA_GP):
            eng = nc.sync if g < 10 else nc.scalar
            eng.dma_start(
                out=xsb[:, g * DMA_GP : (g + 1) * DMA_GP, :],
                in_=xr[:, g * DMA_GP : (g + 1) * DMA_GP, :],
            )

        # --- PE warm-up: data-independent dummy matmuls fill the DMA
        # completion-receipt dead window so the PE clock (HAM p-state)
        # reaches full rate before real work arrives; their results are
        # never used
        warm = const.tile([P, C], FP)
        nc.vector.memset(warm[:], 1.0)
        wps = ps_big.tile([C, C], FP, tag="pt", bufs=3)
        for _ in range(8):
            nc.tensor.matmul(wps[:], warm[:], warm[:])
        nc.vector.tensor_copy(warm[0:C, :], wps[:])  # keep the tile "read"

        # --- [G|m] accumulation, column-packed ----------------------------
        # Distinct PSUM banks per group: start=True clears has_written
        # bank-wide, so interleaved groups must not share a bank.
        # gm accumulators share PSUM slots with the final-phase po tiles
        # (disjoint lifetimes), freeing banks for triple buffering
        gm_a = ps_big.tile([P, W], FP, tag="po", bufs=3)
        gm_b = ps_big.tile([P, W], FP, tag="po", bufs=3)
        # forward transposes of the own half interleave with G
        # accumulation (they only need x, not the chain)
        xt8s = []
        pt = None
        for b in range(NPAIR):
            st, sp = b == 0, b == NPAIR - 1
            # even: rows 0-63 = [m_e | G_e]
            nc.tensor.matmul(
                gm_a[0:C, :], xsb[:, b, 1 : 1 + C], xsb[:, b, 0:W],
                start=st, stop=sp, tile_position=(0, 0),
            )
            # odd: rows 64-127 = [G_o | m_o]
            nc.tensor.matmul(
                gm_b[C:P, :], xsb[:, b, 1 + C : 1 + 2 * C],
                xsb[:, b, 1 + C : BLK],
                start=st, stop=sp, tile_position=(0, C),
            )
            if b < HPAIR:
                # two augmented transposes per pair: [1|x_e] -> ones row 0,
                # [x_o|1] -> ones row 64; 2 pairs batch into one PSUM bank
                j2 = b % 2
                if j2 == 0:
                    pt = ps_big.tile([W, 4, P], FP, tag="pt", bufs=3)
                nc.tensor.transpose(pt[:, 2 * j2, :], xsb[:, b, 0:W], ident)
                nc.tensor.transpose(
                    pt[:, 2 * j2 + 1, :], xsb[:, b, W:BLK], ident
                )
                if j2 == 1:
                    xt4 = sb_t.tile([W, 4, P], FP, tag="xt4", bufs=8)
                    nc.vector.tensor_copy(xt4[:], pt[:])
                    xt8s.append(xt4)
        gmsb = const.tile([P, W], FP)
        nc.vector.tensor_copy(gmsb[0:C, :], gm_a[0:C, :])
        nc.scalar.activation(gmsb[C:P, :], gm_b[C:P, :], AF.Identity)

        # fold top+bottom into G [64,64] and m [64,1] (separate banks)
        f_g = ps_g.tile([C, C], FP, tag="chain")
        nc.tensor.matmul(f_g[:], e_top, gmsb[:, 1 : 1 + C], start=True, stop=False)
        nc.tensor.matmul(f_g[:], e_bot, gmsb[:, 0:C], start=False, stop=True)
        f_m = ps_g.tile([C, 1], FP, tag="chain")
        nc.tensor.matmul(f_m[:], e_top, gmsb[:, 0:1], start=True, stop=False)
        nc.tensor.matmul(f_m[:], e_bot, gmsb[:, C : C + 1], start=False, stop=True)

        # assemble Gt (65 x 65): [[G, m], [m^T, NROWS]]
        gt_sb = const.tile([W, W], FP)
        nc.vector.tensor_copy(gt_sb[0:C, 0:C], f_g[:])
        nc.scalar.activation(gt_sb[0:C, C:W], f_m[:], AF.Identity)
        mt_ps = ps_g.tile([1, C], FP, tag="chain")
        nc.tensor.transpose(mt_ps[:], gt_sb[0:C, C:W], id64)
        nc.vector.tensor_copy(gt_sb[C:W, 0:C], mt_ps[:])
        nc.vector.memset(gt_sb[C:W, C:W], float(N))

        # --- chain: T1 = Gt Wtv^T, then Mmat/c doubled on both halves -----
        # Host folds a Wq^T Wtk into uts and a Wtk^T bq into uqv, so
        # Mmat = Ww^T + uts^T T1 and c = T1^T uqv + bw.
        t1_ps = ps_g.tile([W, C], FP, tag="chain")
        nc.tensor.matmul(t1_ps[:], gt_sb[:], wvta)
        t1_sb = const.tile([W, C], FP)
        nc.vector.tensor_copy(t1_sb[:], t1_ps[:])
        # one matmul gives [Mmat-pre; c_row-pre]; one add applies Ww^T/bw.
        # The result IS Mt_odd = [Mmat; c_row]; Mt_even = [c_row; Mmat] is
        # a cyclic row shift done with one matmul against cyc.
        acr_ps = ps_g.tile([W, C], FP, tag="chain")
        nc.tensor.matmul(acr_ps[:], utq, t1_sb[:])
        m_od = const.tile([W, C], FP)
        nc.vector.tensor_add(m_od[:], acr_ps[:], wwbw)
        me_ps = ps_g.tile([W, C], FP, tag="chain")
        nc.tensor.matmul(me_ps[:], cyc, m_od[:])
        m_ev = const.tile([W, C], FP)
        nc.vector.tensor_copy(m_ev[:], me_ps[:])

        # --- own half: out = gelu(xt @ Mt) directly from PSUM ---------
        osb = const.tile([P, HPAIR, 2 * C], FP)
        orr = out_d[:].rearrange("(b p j) c -> p b (j c)", p=P, j=2)
        for g in range(8):  # 8 groups x 2 pairs (4 row-tiles)
            xt4 = xt8s[g]
            po = ps_big.tile([P, 4, C], FP, tag="po", bufs=3)
            # odd tiles first: m_od is ready two chain hops before m_ev
            for j in (1, 3, 0, 2):
                nc.tensor.matmul(
                    po[:, j, :], xt4[:, j, :],
                    m_ev[:] if j % 2 == 0 else m_od[:],
                )
            nc.scalar.activation(
                osb[:, 2 * g : 2 * g + 2, :].rearrange("p a c -> p (a c)"),
                po[:].rearrange("p a c -> p (a c)"),
                act_fn,
            )
            if g % 2 == 1:
                # one 256KB out-DMA per two groups: the SP ring issues
                # serially, and fewer/bigger transfers drain its queue
                # sooner at the tail
                nc.sync.dma_start(
                    out=orr[:, 2 * g - 2 : 2 * g + 2, :],
                    in_=osb[:, 2 * g - 2 : 2 * g + 2, :],
                )

    nc.compile()
    return nc


_NC_CACHE = None


def _get_nc() -> bass.Bass:
    global _NC_CACHE
    if _NC_CACHE is None:
        _NC_CACHE = build_nc()
    return _NC_CACHE


def make_wpk(inputs: dict) -> np.ndarray:
    Wq, Wk, Wv, Ww = (np.asarray(inputs[k], np.float32) for k in ("Wq", "Wk", "Wv", "Ww"))
    bq, bk, bv, bw = (np.asarray(inputs[k], np.float32) for k in ("bq", "bk", "bv", "bw"))
    wtk = np.concatenate([Wk, bk[:, None]], axis=1)          # [64, 65]
    um = (ALPHA * (Wq.T @ wtk)).astype(np.float32)           # [64, 65]
    uq = (ALPHA * (wtk.T @ bq)).astype(np.float32)           # [65]
    wpk = np.zeros((P, WPK_F), np.float32)
    wpk[0:C, WPK_VT : WPK_VT + C] = Wv.T
    wpk[C, WPK_VT : WPK_VT + C] = bv
    wpk[0:W, WPK_UTQ : WPK_UTQ + C] = um.T
    wpk[0:W, WPK_UTQ + C] = uq
    wpk[0:C, WPK_WB : WPK_WB + C] = Ww.T
    wpk[C, WPK_WB : WPK_WB + C] = bw
    wpk[np.arange(W), WPK_CYC + (np.arange(W) + 1) % W] = 1.0  # cyc
    wpk[:, WPK_ID : WPK_ID + P] = np.eye(P, dtype=np.float32)
    return wpk


def make_in_maps(inputs: dict) -> list[dict]:
    x = np.ascontiguousarray(np.asarray(inputs["x"], dtype=np.float32))
    wpk = np.ascontiguousarray(make_wpk(inputs))
    in_maps = []
    for c in range(NCORES):
        b, h = c // 2, c % 2
        if h == 0:
            xb = x[b]
        else:
            xb = np.concatenate([x[b, N // 2 :], x[b, : N // 2]], axis=0)
        arr = np.ones((P, NPAIR, BLK), np.float32)
        # row(p, pair, j) = pair*256 + 2p + j
        arr[:, :, 1 : 1 + 2 * C] = (
            xb.reshape(NPAIR, P, 2 * C).transpose(1, 0, 2)
        )
        in_maps.append(
            dict(xb=np.ascontiguousarray(arr.reshape(P, NPAIR * BLK)), wpk=wpk)
        )
    return in_maps


def kernel(**inputs) -> np.ndarray:
    nc = _get_nc()
    in_maps = make_in_maps(inputs)
    res = run_bass_kernel_spmd(nc, in_maps, list(range(NCORES)))
    out = np.empty((B, N, C), np.float32)
    for c in range(NCORES):
        b, h = c // 2, c % 2
        out[b, h * (N // 2) : (h + 1) * (N // 2)] = res.results[c]["out"]
    return out
